# revision 59
# baseline (speedup 1.0000x reference)
"""Trainium2 Bass kernel for FusionResidualStabilizer.

reference:
    xn = x / (||x||+eps); r = x - xn
    y  = x + 0.1*(r @ R1 + tanh(r @ R2))
    out = y / (||y||+eps)

Key algebra: r = s*x with per-row scalar s = 1 - 1/||x||, so
    r @ R = s * (x @ R)   (row scale moves past the matmul)
and the final normalization is scale invariant, so with z = 10*y:
    z = (10*x) + s*(x@R1) + tanh(s*(x@R2));  out = z/||z||

Distribution: pure data parallel over the 16384 tokens -> 2048 tokens
per core on 8 cores; R1/R2 replicated.

Host passes per core:
  x  : f32 [2048, 2048] = 10 * x_shard (token major, epilogue + norms)
  xt : fp8e4 [16,128,16,128] = 8 * x_shard transposed tiles (stationary)
  w  : fp8e4 [2, 16, 128, 2048] = 64 * [R1, R2] (moving operand)
The fp8 scales keep values in e4m3's normal range; the epilogue's
per-row scale folds them back out. Matmuls run fp8 DoubleRow (2x).
"""

import sys
import types

import numpy as np
import ml_dtypes

import concourse.bacc as bacc
import concourse.tile as tile
from concourse import mybir
from concourse.bass_utils import run_bass_kernel_spmd

# If BASS_TRACE is set but the image's antenv lacks axon_hooks,
# run_bass_kernel_spmd would crash importing it. Provide a no-op shim so
# tracing degrades gracefully instead.
try:
    import antenv.axon_hooks  # noqa: F401
except ImportError:
    _hooks = types.ModuleType("antenv.axon_hooks")
    _hooks._hook = None
    _hooks.set_axon_ntff_profile_hook = lambda h: setattr(_hooks, "_hook", h)
    _hooks.get_axon_ntff_profile_hook = lambda: _hooks._hook
    sys.modules["antenv.axon_hooks"] = _hooks

DIM = 2048
N_CORES = 8
T_LOCAL = 2048  # tokens per core
TT = T_LOCAL // 128  # 16 token tiles per core
KC = DIM // 128  # 16 contraction chunks
W_SCALE = 64.0  # host pre-scale on weights (keeps fp8 out of subnormals)
X_SCALE = 8.0  # host pre-scale on xt (fp8 stationary)

F32 = mybir.dt.float32
BF16 = mybir.dt.bfloat16
FP8 = mybir.dt.float8e4

LAST_RESULT = None  # BassKernelResults of the most recent run (for test.py)
_NC_CACHE = {}


def _rsqrt(nc, pool, a, tag, a0, iters=2):
    """rsqrt(a) for a [128,1] f32 tile on DVE via Newton iteration seeded
    with the constant rsqrt(a0) (a is statistically close to a0 here: row
    norms of unit-normal data). Keeps Sqrt off ACT so the activation table
    never switches away from the Square/Tanh set. Rel err ~1e-4 even for
    rows 15 sigma off the expected norm."""
    OP = mybir.AluOpType
    y0 = 1.0 / (a0 ** 0.5)
    y = pool.tile([128, 1], mybir.dt.float32, tag=tag)
    t = pool.tile([128, 1], mybir.dt.float32, tag=tag + "t")
    g = nc.vector
    # first Newton step folded with the constant seed: y = 1.5*y0 - 0.5*y0^3*a
    g.tensor_scalar(y[:], a[:], -0.5 * y0 ** 3, 1.5 * y0, OP.mult, OP.add)
    for _ in range(iters):
        # y *= 1.5 - 0.5*a*y^2
        g.tensor_tensor(t[:], y[:], y[:], OP.mult)
        g.tensor_tensor(t[:], t[:], a[:], OP.mult)
        g.tensor_scalar(t[:], t[:], -0.5, 1.5, OP.mult, OP.add)
        g.tensor_tensor(y[:], y[:], t[:], OP.mult)
    return y


def _build_nc():
    nc = bacc.Bacc(
        "TRN2", target_bir_lowering=False, debug=False, num_devices=N_CORES
    )
    x_ext = nc.declare_dram_parameter("x", [T_LOCAL, DIM], F32, isOutput=False)
    xt_ext = nc.declare_dram_parameter("xt", [TT, 128, KC, 128], FP8, isOutput=False)
    w_ext = nc.declare_dram_parameter("w", [2, KC, 128, DIM], FP8, isOutput=False)
    out_ext = nc.declare_dram_parameter("out", [T_LOCAL, DIM], F32, isOutput=True)

    AF = mybir.ActivationFunctionType
    OP = mybir.AluOpType

    with tile.TileContext(nc) as tc:
        with (
            tc.tile_pool(name="wp", bufs=1) as wpool,
            tc.tile_pool(name="xtp", bufs=4) as xtpool,
            tc.tile_pool(name="xp", bufs=4) as xpool,
            tc.tile_pool(name="zp", bufs=2) as zpool,
            tc.tile_pool(name="scrp", bufs=2) as scrpool,
            tc.tile_pool(name="op", bufs=4) as opool,
            tc.tile_pool(name="smp", bufs=4) as smpool,
            tc.tile_pool(name="psp", bufs=1, space="PSUM") as pspool,
        ):
            loaded = {}

            def load_tile(tt):
                x_t = xpool.tile([128, DIM], F32, tag="x")
                xt_t = xtpool.tile([128, KC, 128], FP8, tag="xt")
                nc.sync.dma_start(xt_t[:], xt_ext[tt, :, :, :])
                nc.sync.dma_start(x_t[:], x_ext[tt * 128:(tt + 1) * 128, :])
                loaded[tt] = (x_t, xt_t)

            # startup critical path: first matmuls need xt0 + w[:, k=0..1]
            # only. Dispatch them from three different queue engines in
            # parallel; x0 (epilogue-only) stays off the critical window.
            x_t0 = xpool.tile([128, DIM], F32, tag="x")
            xt_t0 = xtpool.tile([128, KC, 128], FP8, tag="xt")
            nc.gpsimd.dma_start(xt_t0[:], xt_ext[0, :, :, :])
            nc.scalar.dma_start(x_t0[:], x_ext[0:128, :])
            loaded[0] = (x_t0, xt_t0)
            # PE warm-up: junk matmuls with no DMA deps start right after the
            # preamble and keep the HAM activity window busy, so the real
            # stream begins at 2.4GHz instead of ramping from 1.2GHz.
            scr_w = scrpool.tile([128, DIM], BF16, tag="scr")
            nc.vector.memset(scr_w[:, 0:512], 0.0)
            uw = pspool.tile([128, 1024], F32, tag="u10")
            for _ in range(16):
                nc.tensor.matmul(
                    uw[:, 0:512], scr_w[:, 0:128], scr_w[:, 0:512],
                    start=True, stop=True,
                )

            w_sb = wpool.tile([128, 2, KC, DIM], FP8, tag="w")
            # k=0..1 as k-pair chunks quartered by n, in n order: the c=0
            # matmul for bank q needs exactly chunk (i, q), so the first
            # matmul unblocks after two small dispatches
            for q in range(4):
                qs = slice(q * 512, (q + 1) * 512)
                for i in range(2):
                    nc.sync.dma_start(
                        w_sb[:, i, 0:2, qs],
                        w_ext[i, 0:2, :, qs].rearrange("k p n -> p k n"),
                    )
            # k>=2 per k-pair: completion granularity matches the matmul
            # groups' consumption order
            for k in range(2, KC, 2):
                for i in range(2):
                    nc.sync.dma_start(
                        w_sb[:, i, k:k + 2, :],
                        w_ext[i, k:k + 2, :, :].rearrange("k p n -> p k n"),
                    )

            for tt in range(TT):
                if tt not in loaded:
                    load_tile(tt)
                x_t, xt_t = loaded.pop(tt)

                # row scale: sef = (1 - 10/||10x||) / (W*X) = s / (W*X)
                scr = scrpool.tile([128, DIM], BF16, tag="scr")
                ss = smpool.tile([128, 1], F32, tag="ss")
                nc.scalar.activation(scr[:], x_t[:], AF.Square, accum_out=ss[:])
                inv = _rsqrt(nc, smpool, ss, tag=f"inv{tt % 2}", a0=100.0 * DIM)
                sef = smpool.tile([128, 1], F32, tag="sef")
                wx = W_SCALE * X_SCALE
                nc.vector.tensor_scalar(
                    sef[:], inv[:], -10.0 / wx, 1.0 / wx, OP.mult, OP.add
                )

                zb = zpool.tile([128, DIM], F32, tag="zb")
                # two d2-halves so psum banks pipeline across tiles
                for h in range(2):
                    hs = slice(h * 1024, (h + 1) * 1024)
                    u1 = pspool.tile([128, 1024], F32, tag=f"u1{h}")
                    u2 = pspool.tile([128, 1024], F32, tag=f"u2{h}")
                    DR = mybir.MatmulPerfMode.DoubleRow
                    for c in range(KC // 2):
                        lhs = xt_t[:, 2 * c:2 * c + 2, :]
                        for j in range(2):
                            js = slice(j * 512, (j + 1) * 512)
                            n0 = h * 1024 + j * 512
                            nc.tensor.matmul(
                                u1[:, js], lhs, w_sb[:, 0, 2 * c:2 * c + 2, n0:n0 + 512],
                                start=(c == 0), stop=(c == KC // 2 - 1),
                                perf_mode=DR,
                            )
                            nc.tensor.matmul(
                                u2[:, js], lhs, w_sb[:, 1, 2 * c:2 * c + 2, n0:n0 + 512],
                                start=(c == 0), stop=(c == KC // 2 - 1),
                                perf_mode=DR,
                            )
                    # zb_h = u1*sef ; u2 <- tanh(u2*sef) ; zb_h += u2 ;
                    # zb_h += 10x_h ; zz_h = sum(zb_h^2)  (all per-half so
                    # half 0's chain hides under half 1's matmuls; the very
                    # last half is the only exposed chain, so quarter it)
                    nq = 4 if (tt == TT - 1 and h == 1) else 1
                    qw = 1024 // nq
                    zzqs = []
                    for q in range(nq):
                        qs = slice(h * 1024 + q * qw, h * 1024 + (q + 1) * qw)
                        us = slice(q * qw, (q + 1) * qw)
                        nc.vector.tensor_scalar(zb[:, qs], u1[:, us], sef[:], None, OP.mult)
                        nc.scalar.activation(u2[:, us], u2[:, us], AF.Tanh, scale=sef[:])
                        nc.vector.tensor_tensor(zb[:, qs], zb[:, qs], u2[:, us], OP.add)
                        nc.vector.tensor_tensor(zb[:, qs], zb[:, qs], x_t[:, qs], OP.add)
                        zzq = smpool.tile([128, 1], F32, tag=f"zz{h}{q}")
                        nc.scalar.activation(scr[:, qs], zb[:, qs], AF.Square, accum_out=zzq[:])
                        zzqs.append(zzq)
                    zzh = zzqs[0]
                    for qi in range(1, nq):
                        nxt = smpool.tile([128, 1], F32, tag=f"zzm{h}{qi}")
                        nc.vector.tensor_tensor(nxt[:], zzh[:], zzqs[qi][:], OP.add)
                        zzh = nxt
                    if h == 0:
                        zz0 = zzh
                # out = z/||z||
                zz = smpool.tile([128, 1], F32, tag="zz")
                nc.vector.tensor_tensor(zz[:], zz0[:], zzh[:], OP.add)
                ziv = _rsqrt(nc, smpool, zz, tag=f"ziv{tt % 2}", a0=100.0 * DIM, iters=1)
                o_t = opool.tile([128, DIM], F32, tag="o")
                for h in range(2):
                    hs = slice(h * 1024, (h + 1) * 1024)
                    nc.vector.tensor_scalar(o_t[:, hs], zb[:, hs], ziv[:], None, OP.mult)
                    nc.scalar.dma_start(
                        out_ext[tt * 128:(tt + 1) * 128, hs], o_t[:, hs]
                    )

    nc.compile()
    return nc


def kernel(x, R1, R2):
    global LAST_RESULT
    x = np.asarray(x)
    in_dtype = x.dtype
    fp8_np = ml_dtypes.float8_e4m3
    xf = np.ascontiguousarray(x, dtype=np.float32).reshape(N_CORES * T_LOCAL, DIM)
    w = np.stack([np.asarray(R1), np.asarray(R2)]).astype(np.float32) * np.float32(W_SCALE)
    w = w.astype(fp8_np).reshape(2, KC, 128, DIM)

    in_maps = []
    for c in range(N_CORES):
        sh = xf[c * T_LOCAL:(c + 1) * T_LOCAL]  # [2048, 2048]
        x_h = np.ascontiguousarray(sh * np.float32(10.0))
        x4 = (sh * np.float32(X_SCALE)).reshape(TT, 128, KC, 128)  # [tt, t, k, p]
        xt = np.ascontiguousarray(x4.transpose(0, 3, 2, 1)).astype(fp8_np)
        in_maps.append({"x": x_h, "xt": xt, "w": w})

    if "nc" not in _NC_CACHE:
        _NC_CACHE["nc"] = _build_nc()
    nc = _NC_CACHE["nc"]

    res = run_bass_kernel_spmd(nc, in_maps, list(range(N_CORES)))
    LAST_RESULT = res
    out = np.concatenate([res.results[i]["out"] for i in range(N_CORES)], axis=0)
    return out.reshape(x.shape).astype(in_dtype, copy=False)


# revision 60
# speedup vs baseline: 1.0058x; 1.0058x over previous
"""Trainium2 Bass kernel for FusionResidualStabilizer.

reference:
    xn = x / (||x||+eps); r = x - xn
    y  = x + 0.1*(r @ R1 + tanh(r @ R2))
    out = y / (||y||+eps)

Key algebra: r = s*x with per-row scalar s = 1 - 1/||x||, so
    r @ R = s * (x @ R)   (row scale moves past the matmul)
and the final normalization is scale invariant, so with z = 10*y:
    z = (10*x) + s*(x@R1) + tanh(s*(x@R2));  out = z/||z||

Distribution: pure data parallel over the 16384 tokens -> 2048 tokens
per core on 8 cores; R1/R2 replicated.

Host passes per core:
  x  : f32 [2048, 2048] = 10 * x_shard (token major, epilogue + norms)
  xt : fp8e4 [16,128,16,128] = 8 * x_shard transposed tiles (stationary)
  w  : fp8e4 [2, 16, 128, 2048] = 64 * [R1, R2] (moving operand)
The fp8 scales keep values in e4m3's normal range; the epilogue's
per-row scale folds them back out. Matmuls run fp8 DoubleRow (2x).
"""

import sys
import types

import numpy as np
import ml_dtypes

import concourse.bacc as bacc
import concourse.tile as tile
from concourse import mybir
from concourse.bass_utils import run_bass_kernel_spmd

# If BASS_TRACE is set but the image's antenv lacks axon_hooks,
# run_bass_kernel_spmd would crash importing it. Provide a no-op shim so
# tracing degrades gracefully instead.
try:
    import antenv.axon_hooks  # noqa: F401
except ImportError:
    _hooks = types.ModuleType("antenv.axon_hooks")
    _hooks._hook = None
    _hooks.set_axon_ntff_profile_hook = lambda h: setattr(_hooks, "_hook", h)
    _hooks.get_axon_ntff_profile_hook = lambda: _hooks._hook
    sys.modules["antenv.axon_hooks"] = _hooks

DIM = 2048
N_CORES = 8
T_LOCAL = 2048  # tokens per core
TT = T_LOCAL // 128  # 16 token tiles per core
KC = DIM // 128  # 16 contraction chunks
W_SCALE = 64.0  # host pre-scale on weights (keeps fp8 out of subnormals)
X_SCALE = 8.0  # host pre-scale on xt (fp8 stationary)

F32 = mybir.dt.float32
BF16 = mybir.dt.bfloat16
FP8 = mybir.dt.float8e4

LAST_RESULT = None  # BassKernelResults of the most recent run (for test.py)
_NC_CACHE = {}


def _rsqrt(nc, pool, a, tag, a0, iters=2):
    """rsqrt(a) for a [128,1] f32 tile on DVE via Newton iteration seeded
    with the constant rsqrt(a0) (a is statistically close to a0 here: row
    norms of unit-normal data). Keeps Sqrt off ACT so the activation table
    never switches away from the Square/Tanh set. Rel err ~1e-4 even for
    rows 15 sigma off the expected norm."""
    OP = mybir.AluOpType
    y0 = 1.0 / (a0 ** 0.5)
    y = pool.tile([128, 1], mybir.dt.float32, tag=tag)
    t = pool.tile([128, 1], mybir.dt.float32, tag=tag + "t")
    g = nc.vector
    # first Newton step folded with the constant seed: y = 1.5*y0 - 0.5*y0^3*a
    g.tensor_scalar(y[:], a[:], -0.5 * y0 ** 3, 1.5 * y0, OP.mult, OP.add)
    for _ in range(iters):
        # y *= 1.5 - 0.5*a*y^2
        g.tensor_tensor(t[:], y[:], y[:], OP.mult)
        g.tensor_tensor(t[:], t[:], a[:], OP.mult)
        g.tensor_scalar(t[:], t[:], -0.5, 1.5, OP.mult, OP.add)
        g.tensor_tensor(y[:], y[:], t[:], OP.mult)
    return y


def _build_nc():
    nc = bacc.Bacc(
        "TRN2", target_bir_lowering=False, debug=False, num_devices=N_CORES
    )
    x_ext = nc.declare_dram_parameter("x", [T_LOCAL, DIM], F32, isOutput=False)
    xt_ext = nc.declare_dram_parameter("xt", [TT, 128, KC, 128], FP8, isOutput=False)
    w_ext = nc.declare_dram_parameter("w", [2, KC, 128, DIM], FP8, isOutput=False)
    out_ext = nc.declare_dram_parameter("out", [T_LOCAL, DIM], F32, isOutput=True)

    AF = mybir.ActivationFunctionType
    OP = mybir.AluOpType

    with tile.TileContext(nc) as tc:
        with (
            tc.tile_pool(name="wp", bufs=1) as wpool,
            tc.tile_pool(name="xtp", bufs=4) as xtpool,
            tc.tile_pool(name="xp", bufs=4) as xpool,
            tc.tile_pool(name="zp", bufs=2) as zpool,
            tc.tile_pool(name="scrp", bufs=2) as scrpool,
            tc.tile_pool(name="op", bufs=4) as opool,
            tc.tile_pool(name="smp", bufs=4) as smpool,
            tc.tile_pool(name="psp", bufs=1, space="PSUM") as pspool,
        ):
            loaded = {}

            def load_tile(tt):
                x_t = xpool.tile([128, DIM], F32, tag="x")
                xt_t = xtpool.tile([128, KC, 128], FP8, tag="xt")
                nc.sync.dma_start(xt_t[:], xt_ext[tt, :, :, :])
                nc.sync.dma_start(x_t[:], x_ext[tt * 128:(tt + 1) * 128, :])
                loaded[tt] = (x_t, xt_t)

            # startup critical path: first matmuls need xt0 + w[:, k=0..1]
            # only. Dispatch them from three different queue engines in
            # parallel; x0 (epilogue-only) stays off the critical window.
            x_t0 = xpool.tile([128, DIM], F32, tag="x")
            xt_t0 = xtpool.tile([128, KC, 128], FP8, tag="xt")
            nc.gpsimd.dma_start(xt_t0[:], xt_ext[0, :, :, :])
            nc.scalar.dma_start(x_t0[:], x_ext[0:128, :])
            loaded[0] = (x_t0, xt_t0)
            # PE warm-up: junk matmuls with no DMA deps start right after the
            # preamble and keep the HAM activity window busy, so the real
            # stream begins at 2.4GHz instead of ramping from 1.2GHz.
            scr_w = scrpool.tile([128, DIM], BF16, tag="scr")
            nc.vector.memset(scr_w[:, 0:512], 0.0)
            uw = pspool.tile([128, 1024], F32, tag="u10")
            for _ in range(16):
                nc.tensor.matmul(
                    uw[:, 0:512], scr_w[:, 0:128], scr_w[:, 0:512],
                    start=True, stop=True,
                )

            w_sb = wpool.tile([128, 2, KC, DIM], FP8, tag="w")
            # k=0..1 as k-pair chunks quartered by n, in n order: the c=0
            # matmul for bank q needs exactly chunk (i, q), so the first
            # matmul unblocks after two small dispatches
            for q in range(4):
                qs = slice(q * 512, (q + 1) * 512)
                for i in range(2):
                    nc.sync.dma_start(
                        w_sb[:, i, 0:2, qs],
                        w_ext[i, 0:2, :, qs].rearrange("k p n -> p k n"),
                    )
            # k>=2 per k-pair: completion granularity matches the matmul
            # groups' consumption order
            for k in range(2, KC, 2):
                for i in range(2):
                    nc.sync.dma_start(
                        w_sb[:, i, k:k + 2, :],
                        w_ext[i, k:k + 2, :, :].rearrange("k p n -> p k n"),
                    )

            for tt in range(TT):
                if tt not in loaded:
                    load_tile(tt)
                x_t, xt_t = loaded.pop(tt)

                # row scale: sef = (1 - 10/||10x||) / (W*X) = s / (W*X)
                scr = scrpool.tile([128, DIM], BF16, tag="scr")
                ss = smpool.tile([128, 1], F32, tag="ss")
                nc.scalar.activation(scr[:], x_t[:], AF.Square, accum_out=ss[:])
                inv = _rsqrt(nc, smpool, ss, tag=f"inv{tt % 2}", a0=100.0 * DIM)
                sef = smpool.tile([128, 1], F32, tag="sef")
                wx = W_SCALE * X_SCALE
                nc.vector.tensor_scalar(
                    sef[:], inv[:], -10.0 / wx, 1.0 / wx, OP.mult, OP.add
                )

                zb = zpool.tile([128, DIM], F32, tag="zb")
                # two d2-halves so psum banks pipeline across tiles
                for h in range(2):
                    hs = slice(h * 1024, (h + 1) * 1024)
                    u1 = pspool.tile([128, 1024], F32, tag=f"u1{h}")
                    u2 = pspool.tile([128, 1024], F32, tag=f"u2{h}")
                    DR = mybir.MatmulPerfMode.DoubleRow
                    for c in range(KC // 2):
                        lhs = xt_t[:, 2 * c:2 * c + 2, :]
                        for j in range(2):
                            js = slice(j * 512, (j + 1) * 512)
                            n0 = h * 1024 + j * 512
                            nc.tensor.matmul(
                                u1[:, js], lhs, w_sb[:, 0, 2 * c:2 * c + 2, n0:n0 + 512],
                                start=(c == 0), stop=(c == KC // 2 - 1),
                                perf_mode=DR,
                            )
                            nc.tensor.matmul(
                                u2[:, js], lhs, w_sb[:, 1, 2 * c:2 * c + 2, n0:n0 + 512],
                                start=(c == 0), stop=(c == KC // 2 - 1),
                                perf_mode=DR,
                            )
                    # zb_h = u1*sef ; u2 <- tanh(u2*sef) ; zb_h += u2 ;
                    # zb_h += 10x_h ; zz_h = sum(zb_h^2)  (all per-half so
                    # half 0's chain hides under half 1's matmuls; the very
                    # last half is the only exposed chain, so quarter it)
                    nq = 2 if (tt == TT - 1 and h == 1) else 1
                    qw = 1024 // nq
                    zzqs = []
                    for q in range(nq):
                        qs = slice(h * 1024 + q * qw, h * 1024 + (q + 1) * qw)
                        us = slice(q * qw, (q + 1) * qw)
                        nc.vector.tensor_scalar(zb[:, qs], u1[:, us], sef[:], None, OP.mult)
                        nc.scalar.activation(u2[:, us], u2[:, us], AF.Tanh, scale=sef[:])
                        nc.vector.tensor_tensor(zb[:, qs], zb[:, qs], u2[:, us], OP.add)
                        nc.vector.tensor_tensor(zb[:, qs], zb[:, qs], x_t[:, qs], OP.add)
                        zzq = smpool.tile([128, 1], F32, tag=f"zz{h}{q}")
                        nc.scalar.activation(scr[:, qs], zb[:, qs], AF.Square, accum_out=zzq[:])
                        zzqs.append(zzq)
                    zzh = zzqs[0]
                    for qi in range(1, nq):
                        nxt = smpool.tile([128, 1], F32, tag=f"zzm{h}{qi}")
                        nc.vector.tensor_tensor(nxt[:], zzh[:], zzqs[qi][:], OP.add)
                        zzh = nxt
                    if h == 0:
                        zz0 = zzh
                # out = z/||z||
                zz = smpool.tile([128, 1], F32, tag="zz")
                nc.vector.tensor_tensor(zz[:], zz0[:], zzh[:], OP.add)
                ziv = _rsqrt(nc, smpool, zz, tag=f"ziv{tt % 2}", a0=100.0 * DIM, iters=1)
                o_t = opool.tile([128, DIM], F32, tag="o")
                for h in range(2):
                    hs = slice(h * 1024, (h + 1) * 1024)
                    nc.vector.tensor_scalar(o_t[:, hs], zb[:, hs], ziv[:], None, OP.mult)
                    nc.scalar.dma_start(
                        out_ext[tt * 128:(tt + 1) * 128, hs], o_t[:, hs]
                    )

    nc.compile()
    return nc


def kernel(x, R1, R2):
    global LAST_RESULT
    x = np.asarray(x)
    in_dtype = x.dtype
    fp8_np = ml_dtypes.float8_e4m3
    xf = np.ascontiguousarray(x, dtype=np.float32).reshape(N_CORES * T_LOCAL, DIM)
    w = np.stack([np.asarray(R1), np.asarray(R2)]).astype(np.float32) * np.float32(W_SCALE)
    w = w.astype(fp8_np).reshape(2, KC, 128, DIM)

    in_maps = []
    for c in range(N_CORES):
        sh = xf[c * T_LOCAL:(c + 1) * T_LOCAL]  # [2048, 2048]
        x_h = np.ascontiguousarray(sh * np.float32(10.0))
        x4 = (sh * np.float32(X_SCALE)).reshape(TT, 128, KC, 128)  # [tt, t, k, p]
        xt = np.ascontiguousarray(x4.transpose(0, 3, 2, 1)).astype(fp8_np)
        in_maps.append({"x": x_h, "xt": xt, "w": w})

    if "nc" not in _NC_CACHE:
        _NC_CACHE["nc"] = _build_nc()
    nc = _NC_CACHE["nc"]

    res = run_bass_kernel_spmd(nc, in_maps, list(range(N_CORES)))
    LAST_RESULT = res
    out = np.concatenate([res.results[i]["out"] for i in range(N_CORES)], axis=0)
    return out.reshape(x.shape).astype(in_dtype, copy=False)
